# revision 25
# baseline (speedup 1.0000x reference)
"""Causal multi-head attention (RoPE) TRN2 Bass kernel.

Problem: x[2,2048,2048] fp32, Wq/Wk/Wv/Wo [2048,2048], 16 heads, d_k=128,
causal softmax attention with interleaved RoPE, out = attn_out @ Wo.

Sharding (8 cores): core = b*4 + g handles batch b and head group g
(4 heads = 512 feature columns). Wq/Wk/Wv split column-wise, Wo row-wise;
the "all-reduce" after the output projection is done on the host by summing
the 4 partial outputs per batch (gather/unshard step).

Device kernel (per core). All matmul operands are bf16 (PSUM accumulation
stays fp32): bf16 runs at the same 1 row/cycle PE rate as fp32r but gets
the compiler-automatic Fast Weight Load path, so the per-matmul LDWEIGHTS
(~128 cycles, fully serialized for fp32r = 22% PE overhead) is pulled
ahead into the background weight buffer and hidden. bf16 also halves DMA
and SBUF so all four weight matrices stay resident across chunks (the
fp32r version re-streamed Wq/Wk/Wv every chunk: 90MB total DMA vs ~30MB).

Per 512-row chunk j:
  section 1 (merged, PE-dense): QT/KT = (x @ Wq/Wk)^T via lhsT=W tiles,
    rhs=xT, with RoPE fused on VectorE; V = x @ Wv (natural layout, lhsT=xT
    tiles) interleaved two k-steps per projection group; the PREVIOUS chunk's
    output projection (O @ Wo) interleaved as well so its PSUM drains and
    copies hide under dense matmuls.
  section 2: causal attention for q-tile j, scores computed transposed
    (S^T[k,q]) so softmax weights feed attn@V without any transposes; exp on
    ScalarE (no max subtraction needed: scores are O(5)); the o accumulation
    matmuls trail the S matmuls by 4 slots in one flat software-pipelined
    stream across all four heads. Softmax row sums do NOT use per-block
    ones-matmuls (that would be 160 extra 512-row matmuls on the PE):
    instead e blocks are accumulated elementwise into a fp32 e_sum on
    GpSimd/VectorE (alternating, so neither engine becomes critical) and a
    single ones-lhsT matmul per (head, chunk) reduces e_sum over partitions.
    Masked-out leading columns of diagonal blocks are simply never computed
    (the exp, o-matmul, and e_sum-add all restrict to the live column
    range), so no zero-fill pass exists. The softmax reciprocal uses the
    fast custom-DVE approximation (~18 bits, 5x faster than the exact
    InstReciprocal). The PE is pre-warmed with dummy matmuls during the
    initial DMA wait so the HAM clock gate lifts to 2.4 GHz before real
    work arrives.

RoPE pair trick: scores are invariant under any permutation of d_k applied
to both Q and K, so W columns are permuted per head to [even..., odd...] on
the host; the rotate pairs then live 64 partitions apart (two plain
partition-offset copies instead of an interleaved shuffle), and cosT/sinT
are permuted/sign-baked to match.
"""

import math
import sys

sys.path.insert(0, "/opt/trn_rl_repo")

import ml_dtypes
import numpy as np

D_MODEL = 2048
SEQ = 2048
BATCH = 2
N_CORES = 8
HEADS_PER_CORE = 4
GCOLS = HEADS_PER_CORE * 128  # 512 feature columns per core
KB = D_MODEL // 128  # 16 contraction blocks
N_CHUNKS = SEQ // 512  # 4
SCALE = 1.0 / math.sqrt(128.0)

_CACHE = {}


def _build_program():
    import concourse.mybir as mybir
    import concourse.tile as tile
    from concourse import bacc

    F = mybir.dt.float32
    FR = mybir.dt.float32r
    BF = mybir.dt.bfloat16
    AF = mybir.ActivationFunctionType

    nc = bacc.Bacc("TRN2", target_bir_lowering=False, debug=False,
                   num_devices=N_CORES)

    xT_d = nc.dram_tensor("xT", (D_MODEL, SEQ), BF, kind="ExternalInput").ap()
    Wq_d = nc.dram_tensor("Wq", (D_MODEL, GCOLS), BF, kind="ExternalInput").ap()
    Wk_d = nc.dram_tensor("Wk", (D_MODEL, GCOLS), BF, kind="ExternalInput").ap()
    Wv_d = nc.dram_tensor("Wv", (D_MODEL, GCOLS), BF, kind="ExternalInput").ap()
    Wo_d = nc.dram_tensor("Wo", (GCOLS, D_MODEL), BF, kind="ExternalInput").ap()
    cosT_d = nc.dram_tensor("cosT", (128, SEQ), F, kind="ExternalInput").ap()
    sinT_d = nc.dram_tensor("sinT", (128, SEQ), F, kind="ExternalInput").ap()
    mask_d = nc.dram_tensor("mask", (128, 896), BF, kind="ExternalInput").ap()
    out_d = nc.dram_tensor("out", (SEQ, D_MODEL), F, kind="ExternalOutput").ap()

    with tile.TileContext(nc) as tc:
        with tc.tile_pool(name="resid", bufs=1) as resid, \
             tc.tile_pool(name="xtp", bufs=2) as xtp, \
             tc.tile_pool(name="csp", bufs=2) as csp, \
             tc.tile_pool(name="qtp", bufs=1) as qtp, \
             tc.tile_pool(name="otp", bufs=1) as otp, \
             tc.tile_pool(name="ep", bufs=10) as ep, \
             tc.tile_pool(name="ropep", bufs=2) as ropep, \
             tc.tile_pool(name="rcp", bufs=1) as rcp, \
             tc.tile_pool(name="outp", bufs=3) as outp, \
             tc.tile_pool(name="psA", bufs=4, space="PSUM") as psA, \
             tc.tile_pool(name="psB", bufs=4, space="PSUM") as psB:

            ones_f = resid.tile([128, 128], F, tag="ones_f")
            nc.vector.memset(ones_f[:], 1.0)
            ones_bf = resid.tile([128, 128], BF, tag="ones_bf")
            nc.vector.tensor_copy(ones_bf[:], ones_f[:])
            mask_sb = resid.tile([128, 128], BF, tag="mask")

            # Pre-warm the PE while the first DMAs are in flight: the HAM
            # clock gate needs ~3.4us of sustained activity to lift the PE
            # from 1.2 to 2.4 GHz, and the initial DMA wait is dead time
            # anyway.
            warm_ps = psA.tile([128, 128], F, tag="flow", name="warm")
            for _ in range(12):
                nc.tensor.matmul(warm_ps[:], ones_bf[:], ones_bf[:],
                                 start=True, stop=True)
            KT = resid.tile([128, HEADS_PER_CORE, SEQ], BF, tag="KT")
            V = resid.tile([128, KB, GCOLS], BF, tag="V")
            wo = resid.tile([128, HEADS_PER_CORE, D_MODEL], BF, tag="wo")
            # resident weights, layout [p=128, ko, cols]
            wq = resid.tile([128, KB, GCOLS], BF, tag="wq")
            wk = resid.tile([128, KB, GCOLS], BF, tag="wk")
            wv = resid.tile([128, KB, GCOLS], BF, tag="wv")

            xT_r = xT_d.rearrange("(ko p) s -> p ko s", p=128)
            Wq_r = Wq_d.rearrange("(ko p) m -> p ko m", p=128)
            Wk_r = Wk_d.rearrange("(ko p) m -> p ko m", p=128)
            Wv_r = Wv_d.rearrange("(ko p) m -> p ko m", p=128)

            def emit_wo_step(jprev, prev_ot, m, n, alt=False):
                ps = psA.tile([128, 512], F, tag="flow", name="wops")
                for c in range(HEADS_PER_CORE):
                    nc.tensor.matmul(
                        ps[:], prev_ot[:, c, m * 128:(m + 1) * 128],
                        wo[:, c, n * 512:(n + 1) * 512],
                        start=(c == 0), stop=(c == 3))
                ob = outp.tile([128, 512], F, tag="ob")
                if alt:
                    nc.scalar.copy(ob[:], ps[:])
                else:
                    nc.vector.tensor_copy(ob[:], ps[:])
                nc.sync.dma_start(
                    out_d[(4 * jprev + m) * 128:(4 * jprev + m + 1) * 128,
                          n * 512:(n + 1) * 512], ob[:])

            def stage_inputs(jj):
                # issue chunk jj's input DMAs (called one chunk ahead so the
                # transfers land before the projections need them)
                sl = slice(jj * 512, (jj + 1) * 512)
                xt_n = xtp.tile([128, KB, 512], BF, tag="xt", name="xt_n")
                nc.sync.dma_start(xt_n[:, 0:8], xT_r[:, 0:8, sl])
                nc.sync.dma_start(xt_n[:, 8:KB], xT_r[:, 8:KB, sl])
                cos_n = csp.tile([128, 512], F, tag="cos", name="cos_n")
                nc.sync.dma_start(cos_n[:], cosT_d[:, sl])
                sin_n = csp.tile([128, 512], F, tag="sin", name="sin_n")
                nc.sync.dma_start(sin_n[:], sinT_d[:, sl])
                return xt_n, cos_n, sin_n

            staged = {}
            prev_ot = None

            for j in range(N_CHUNKS):
                ssl = slice(j * 512, (j + 1) * 512)

                # xT in pieces so the first matmuls only wait on a fraction
                # of the chunk; on j==0 the resident weights are interleaved
                # in the order compute first touches them.
                if j == 0:
                    # fine-grained pieces so the first matmul only waits on
                    # ~a quarter MB, and the k-stream stays ahead of the PE
                    xt = xtp.tile([128, KB, 512], BF, tag="xt")
                    nc.scalar.dma_start(wq[:, 0:2], Wq_r[:, 0:2, 0:GCOLS])
                    nc.sync.dma_start(xt[:, 0:2], xT_r[:, 0:2, ssl])
                    nc.scalar.dma_start(wq[:, 2:4], Wq_r[:, 2:4, 0:GCOLS])
                    nc.sync.dma_start(xt[:, 2:4], xT_r[:, 2:4, ssl])
                    cos_t = csp.tile([128, 512], F, tag="cos")
                    nc.sync.dma_start(cos_t[:], cosT_d[:, ssl])
                    sin_t = csp.tile([128, 512], F, tag="sin")
                    nc.sync.dma_start(sin_t[:], sinT_d[:, ssl])
                    nc.scalar.dma_start(wv[:, 0:2], Wv_r[:, 0:2, 0:GCOLS])
                    nc.scalar.dma_start(wq[:, 4:8], Wq_r[:, 4:8, 0:GCOLS])
                    nc.sync.dma_start(xt[:, 4:8], xT_r[:, 4:8, ssl])
                    nc.scalar.dma_start(wq[:, 8:12], Wq_r[:, 8:12, 0:GCOLS])
                    nc.sync.dma_start(xt[:, 8:12], xT_r[:, 8:12, ssl])
                    nc.scalar.dma_start(wq[:, 12:KB], Wq_r[:, 12:KB, 0:GCOLS])
                    nc.sync.dma_start(xt[:, 12:KB], xT_r[:, 12:KB, ssl])
                    nc.scalar.dma_start(wv[:, 2:6], Wv_r[:, 2:6, 0:GCOLS])
                    nc.scalar.dma_start(wk[:, 0:8], Wk_r[:, 0:8, 0:GCOLS])
                    nc.scalar.dma_start(wv[:, 6:KB], Wv_r[:, 6:KB, 0:GCOLS])
                    nc.scalar.dma_start(wk[:, 8:KB], Wk_r[:, 8:KB, 0:GCOLS])
                    nc.scalar.dma_start(mask_sb[:], mask_d[:, 384:512])
                else:
                    xt, cos_t, sin_t = staged.pop(j)
                if j == 0:
                    staged[1] = stage_inputs(1)
                qt = qtp.tile([128, HEADS_PER_CORE, 512], BF, tag="qt")

                # --- Q/K projections + RoPE (outputs transposed: [d_k, s]),
                # with the V projection's k-steps interleaved between groups
                # so the RoPE VectorE chain gets slack against the PE
                # stream. ---
                vps = [psB.tile([128, 512], F, tag="hold", name=f"vps{m}")
                       for m in range(4)]
                groups = [(qt, True, wq, m) for m in range(HEADS_PER_CORE)]
                groups += [(KT, False, wk, m) for m in range(HEADS_PER_CORE)]
                for g, (dst, is_q, w, m) in enumerate(groups):
                    ps = psA.tile([128, 512], F, tag="flow")
                    for k in range(KB):
                        nc.tensor.matmul(ps[:], w[:, k, m * 128:(m + 1) * 128],
                                         xt[:, k],
                                         start=(k == 0), stop=(k == KB - 1))
                    # two V k-steps per QK group
                    for k in (2 * g, 2 * g + 1):
                        for m2 in range(4):
                            nc.tensor.matmul(
                                vps[m2][:],
                                xt[:, k, m2 * 128:(m2 + 1) * 128], wv[:, k],
                                start=(k == 0), stop=(k == KB - 1))
                    # two Wo output-projection steps for the previous chunk
                    if prev_ot is not None:
                        for t in (2 * g, 2 * g + 1):
                            emit_wo_step(j - 1, prev_ot, t // 4, t % 4)
                    rot = ropep.tile([128, 512], F, tag="rot")
                    nc.vector.tensor_copy(rot[:64, :], ps[64:128, :])
                    nc.vector.tensor_copy(rot[64:128, :], ps[:64, :])
                    out_ap = dst[:, m, :] if is_q else dst[:, m, ssl]
                    nc.vector.tensor_mul(out_ap, ps[:], cos_t[:])
                    nc.vector.tensor_mul(rot[:], rot[:], sin_t[:])
                    nc.vector.tensor_add(out_ap, out_ap, rot[:])
                for m in range(4):
                    nc.any.tensor_copy(V[:, 4 * j + m, :], vps[m][:])

                if 0 < j < N_CHUNKS - 1:
                    staged[j + 1] = stage_inputs(j + 1)

                if j == 0:
                    # Wo is first needed ~100us in; keep it off the startup
                    # critical path but loaded well before the Wo section.
                    nc.scalar.dma_start(
                        wo[:], Wo_d.rearrange("(c p) n -> p c n", p=128))

                # --- causal attention for q-tile j ---
                # kb order per head: diagonal blocks first (their exp+mask
                # chain is the longest), then the fully-unmasked history
                # blocks. One flat software-pipelined stream across all four
                # heads: the rs/o accumulation matmuls for E(i) trail the S
                # matmuls by LAG slots, and the pipeline never drains at head
                # boundaries.
                ot = otp.tile([128, HEADS_PER_CORE, 512], BF, tag="ot")
                diag = list(range(4 * j, 4 * j + 4))
                hist = list(range(4 * j))
                if hist:
                    # interleave: one diagonal block, then a run of history
                    kb_order = []
                    step = max(1, len(hist) // 4)
                    hi = 0
                    for dkb in diag:
                        kb_order.append(dkb)
                        kb_order.extend(hist[hi:hi + step])
                        hi += step
                    kb_order.extend(hist[hi:])
                else:
                    kb_order = diag
                nkb = len(kb_order)
                LAG = 4
                hstate = {}
                pend = []
                norm_q = []

                def flush_one():
                    h, i, kb, e, lo = pend.pop(0)
                    rs_ps, o_ps = hstate[h]
                    nc.tensor.matmul(rs_ps[:, lo:], ones_bf[:], e[:, lo:],
                                     start=(i == 0), stop=(i == nkb - 1),
                                     skip_group_check=True)
                    nc.tensor.matmul(
                        o_ps[:, lo:], V[:, kb, h * 128:(h + 1) * 128],
                        e[:, lo:], start=(i == 0), stop=(i == nkb - 1),
                        skip_group_check=True)
                    if i == nkb - 1:
                        norm_q.append(h)

                def emit_norm():
                    h = norm_q.pop(0)
                    rs_ps, o_ps = hstate[h]
                    rc = rcp.tile([128, 512], F, tag="rc")
                    # narrower pieces in the last chunk so the trailing Wo
                    # matmuls only wait for the slice they read
                    pw = 128 if j == N_CHUNKS - 1 else 256
                    for q0 in range(0, 512, pw):
                        nc.vector.reciprocal_approx_fast(
                            rc[:, q0:q0 + pw], rs_ps[:, q0:q0 + pw])
                        nc.vector.tensor_mul(ot[:, h, q0:q0 + pw],
                                             o_ps[:, q0:q0 + pw],
                                             rc[:, q0:q0 + pw])

                norm_keep = 0 if j == N_CHUNKS - 1 else 1
                for h in range(HEADS_PER_CORE):
                    while len(norm_q) > norm_keep:
                        emit_norm()
                    hstate[h] = (
                        psB.tile([128, 512], F, tag="hold", name=f"rs{h}"),
                        psB.tile([128, 512], F, tag="hold", name=f"o{h}"))
                    for i, kb in enumerate(kb_order):
                        s_ps = psA.tile([128, 512], F, tag="flow")
                        d = kb - 4 * j
                        lo = 128 * d if d > 0 else 0
                        nc.tensor.matmul(
                            s_ps[:, lo:], KT[:, h, kb * 128:(kb + 1) * 128],
                            qt[:, h, lo:], start=True, stop=True)
                        while len(pend) >= LAG:
                            flush_one()
                        e = ep.tile([128, 512], BF, tag="e")
                        if d >= 0:
                            # diagonal block: columns < 128d are fully masked
                            # and never touched downstream; the 128-wide
                            # block at 128d is triangular, the rest is
                            # unmasked.
                            nc.scalar.activation(
                                e[:, lo:], s_ps[:, lo:], AF.Exp,
                                scale=SCALE)
                            nc.vector.tensor_mul(
                                e[:, lo:lo + 128],
                                e[:, lo:lo + 128], mask_sb[:])
                        else:
                            nc.scalar.activation(e[:], s_ps[:], AF.Exp,
                                                 scale=SCALE)
                        pend.append((h, i, kb, e, lo))
                        if i == nkb - 2 and norm_q:
                            emit_norm()
                while pend:
                    flush_one()
                while norm_q:
                    emit_norm()
                prev_ot = ot

            # output projection for the last chunk; copies alternate between
            # VectorE and the now-idle ScalarE to halve the drain tail
            for t in range(16):
                emit_wo_step(N_CHUNKS - 1, prev_ot, t // 4, t % 4,
                             alt=(t % 2 == 1))

    nc.compile()
    return nc


def _get_program():
    if "nc" not in _CACHE:
        _CACHE["nc"] = _build_program()
    return _CACHE["nc"]


def _host_prep(x, token_positions, Wq, Wk, Wv, Wo):
    bf16 = ml_dtypes.bfloat16
    x = np.asarray(x, dtype=np.float32)
    Wq = np.asarray(Wq, dtype=np.float32)
    Wk = np.asarray(Wk, dtype=np.float32)
    Wv = np.asarray(Wv, dtype=np.float32)
    Wo = np.asarray(Wo, dtype=np.float32)
    pos = np.asarray(token_positions).astype(np.float64)

    # RoPE tables in permuted (half-split) layout, transposed to [d_k, s].
    inv = 10000.0 ** (-2.0 * np.arange(64, dtype=np.float64) / 128.0)
    ang = inv[:, None] * pos[None, :]  # [64, S]
    cos_h = np.cos(ang)
    sin_h = np.sin(ang)
    cosT = np.concatenate([cos_h, cos_h], axis=0).astype(np.float32)
    sinT = np.concatenate([-sin_h, sin_h], axis=0).astype(np.float32)

    # half-split permutation of each head's 128 feature columns
    perm = np.concatenate([np.arange(0, 128, 2), np.arange(1, 128, 2)])

    # causal mask strip: mask[p, g] = 1 iff p <= g - 384; diagonal block d
    # (d = kb - 4j) uses columns [384-128d, 896-128d).
    mask = (np.arange(128)[:, None] <= np.arange(896)[None, :] - 384)
    mask = np.ascontiguousarray(mask.astype(bf16))

    def permute_cols(W):  # [2048, 512] -> per-head column permutation
        return np.ascontiguousarray(
            W.reshape(D_MODEL, HEADS_PER_CORE, 128)[:, :, perm].reshape(
                D_MODEL, GCOLS).astype(bf16))

    in_maps = []
    for core in range(N_CORES):
        b, g = divmod(core, 4)
        cols = slice(g * GCOLS, (g + 1) * GCOLS)
        in_maps.append({
            "xT": np.ascontiguousarray(x[b].T.astype(bf16)),
            "Wq": permute_cols(Wq[:, cols]),
            "Wk": permute_cols(Wk[:, cols]),
            "Wv": np.ascontiguousarray(Wv[:, cols].astype(bf16)),
            "Wo": np.ascontiguousarray(Wo[cols, :].astype(bf16)),
            "cosT": cosT,
            "sinT": sinT,
            "mask": mask,
        })
    return in_maps


def run_sharded(x, token_positions, Wq, Wk, Wv, Wo, trace=False, tmpdir=None):
    """Run the SPMD kernel; returns (full_output, BassKernelResults)."""
    from concourse import bass_utils

    nc = _get_program()
    in_maps = _host_prep(x, token_positions, Wq, Wk, Wv, Wo)
    kwargs = {}
    if trace:
        kwargs = {"trace": True, "tmpdir": tmpdir}
    res = bass_utils.run_bass_kernel_spmd(
        nc, in_maps, core_ids=list(range(N_CORES)), **kwargs)
    out = np.empty((BATCH, SEQ, D_MODEL), dtype=np.float32)
    for b in range(BATCH):
        acc = np.zeros((SEQ, D_MODEL), dtype=np.float64)
        for g in range(4):
            acc += res.results[b * 4 + g]["out"]
        out[b] = acc.astype(np.float32)
    return out, res


def kernel(x, token_positions, Wq, Wk, Wv, Wo):
    out, _ = run_sharded(x, token_positions, Wq, Wk, Wv, Wo)
    return out
